# revision 1
# baseline (speedup 1.0000x reference)
"""DepthDC fused kernel for 8 Trainium2 NeuronCores.

Reference computation (N=2, C=64, H=W=256, d=2):
  patches[n,c,k,h,w] = xpad[n,c,h+ki*d, w+kj*d]   (k=3*ki+kj, pad d)
  out1 = sum_k patches * y.reshape(N,C,9,H,W)
  out  = leaky_relu(conv3x3(out1, fuse_w) + fuse_b, 0.2)

Sharding: 8 cores = batch(2) x H-quarters(4). Each core produces a
[64, 64, 256] output slab. Host slices overlapping (haloed, zero-padded)
input slabs per core, so no device collectives are needed.

Per-core layout: the 64 output rows split into two 32-row halves mapped
to SBUF partition halves (partition = c + 64*s). All engines see uniform
[128, F] tiles:
  - DVE: 9 elementwise products prod_k = x_shift(k) * y_k        (fp32)
  - PE:  k-reduction via identity matmul, accumulated in PSUM    (fp32r)
  - PE:  3x3 dense conv as 9 accumulating matmuls over C=64      (fp32r)
  - ACT: PSUM->SBUF copies and the Relu part of leaky-relu
  - DVE: leaky_relu(v) = 0.2*v + 0.8*relu(v) final combine
Work is streamed over 4-row h-chunks with double-buffered y DMA.
"""

import sys

sys.path.insert(0, "/opt/trn_rl_repo")

import numpy as np

import concourse.bass as bass
import concourse.mybir as mybir
import concourse.tile as tile
from concourse import bacc
from concourse.bass_utils import run_bass_kernel_spmd

F32 = mybir.dt.float32
F32R = mybir.dt.float32r
AF = mybir.ActivationFunctionType

N, C, H, W = 2, 64, 256, 256
D = 2  # dilation == pad
NEG_SLOPE = 0.2
NCORES = 8
HB = 64          # output rows per core
HH = 32          # output rows per half
Q = HH + 2       # out1 rows per half (34)
XR = Q + 4       # x rows per half block (38)
XW = W + 2 * D   # padded x width (260)
OW = W + 2       # padded out1 width (258)
RC = 4           # rows per chunk
NRED = 9         # reduce chunks per half: 8 x 4 rows + 1 x 2 rows
NCONV = 8        # conv chunks per half: 8 x 4 rows


def _build_program(loop_reps=None):
    nc = bacc.Bacc("TRN2", target_bir_lowering=False, debug=False,
                   num_devices=NCORES)

    xp_d = nc.dram_tensor("xp", [C, 70, XW], F32, kind="ExternalInput").ap()
    yp_d = nc.dram_tensor("yp", [C * 9, 66, W], F32, kind="ExternalInput").ap()
    wt_d = nc.dram_tensor("wt", [9, 128, 128], F32R, kind="ExternalInput").ap()
    id_d = nc.dram_tensor("ident", [128, 128], F32R, kind="ExternalInput").ap()
    b_d = nc.dram_tensor("bias", [128, 1], F32, kind="ExternalInput").ap()
    out_d = nc.dram_tensor("out", [C, HB, W], F32, kind="ExternalOutput").ap()

    with tile.TileContext(nc) as tc:
        from contextlib import ExitStack
        with ExitStack() as ctx:
            const = ctx.enter_context(tc.tile_pool(name="const", bufs=1))
            y_pool = ctx.enter_context(tc.tile_pool(name="y_pool", bufs=2))
            p_pool = ctx.enter_context(tc.tile_pool(name="p_pool", bufs=4))
            o_pool = ctx.enter_context(tc.tile_pool(name="o_pool", bufs=2))
            v_pool = ctx.enter_context(tc.tile_pool(name="v_pool", bufs=2))
            ps1_pool = ctx.enter_context(
                tc.tile_pool(name="ps1_pool", bufs=2, space="PSUM"))
            ps2_pool = ctx.enter_context(
                tc.tile_pool(name="ps2_pool", bufs=2, space="PSUM"))

            # constants / whole-slab x / whole-slab out1
            w_sb = const.tile([128, 9, 128], F32R, name="w_sb")
            nc.sync.dma_start(w_sb[:], wt_d.rearrange("t p m -> p t m"))
            id_sb = const.tile([128, 128], F32R, name="id_sb")
            nc.sync.dma_start(id_sb[:], id_d)
            b_sb = const.tile([128, 1], F32, name="b_sb")
            nc.sync.dma_start(b_sb[:], b_d)
            x_sb = const.tile([128, XR, XW], F32, name="x_sb")
            nc.sync.dma_start(x_sb[0:64], xp_d[:, 0:XR, :])
            nc.sync.dma_start(x_sb[64:128], xp_d[:, HH:HH + XR, :])
            o1_sb = const.tile([128, Q, OW], F32R, name="o1_sb")
            # zero the conv W-padding columns once (ACT, scale=0 writes 0)
            nc.scalar.activation(o1_sb[:, :, 0:1], o1_sb[:, :, 0:1],
                                 AF.Copy, scale=0.0)
            nc.scalar.activation(o1_sb[:, :, OW - 1:OW],
                                 o1_sb[:, :, OW - 1:OW], AF.Copy, scale=0.0)
            # Wait-merge scratch: one cheap DVE copy per input DMA converts
            # DMA-completion semaphores into DVE program order, so compute
            # instructions never need more than 1 foreign wait sem (the
            # TT-struct wait-slot limit in walrus codegen is tight).
            scr = const.tile([128, 8], F32, name="scr")
            nc.vector.tensor_copy(scr[:, 0:1], x_sb[:, 0, 0:1])
            nc.vector.tensor_copy(scr[:, 1:2], x_sb[:, XR - 1, 0:1])
            nc.vector.tensor_copy(scr[:, 2:3], w_sb[:, 0, 0:1].bitcast(F32))
            nc.vector.tensor_copy(scr[:, 3:4], id_sb[:, 0:1].bitcast(F32))
            nc.vector.tensor_copy(scr[:, 4:5], b_sb[:, 0:1])

            def reduce_chunk(c):
                q0 = RC * c
                rc = min(RC, Q - q0)
                y_t = y_pool.tile([128, 9, RC, W], F32, name="y_t", tag="y_t")
                for s in (0, 1):
                    src = yp_d[:, HH * s + q0: HH * s + q0 + rc, :]
                    src = src.rearrange("(c k) r w -> c k r w", k=9)
                    nc.sync.dma_start(y_t[64 * s:64 * s + 64, :, 0:rc, :], src)
                    nc.vector.tensor_copy(scr[:, 5 + s:6 + s],
                                          y_t[:, 0, 0, 0:1])
                ps1 = ps1_pool.tile([128, RC, W], F32, name="ps1", tag="ps1")
                for k in range(9):
                    ki, kj = divmod(k, 3)
                    p_t = p_pool.tile([128, RC, W], F32R, name="p_t",
                                      tag="p_t")
                    x_view = x_sb[:, q0 + 2 * ki: q0 + 2 * ki + rc,
                                  2 * kj: 2 * kj + W]
                    nc.vector.tensor_mul(p_t[:, 0:rc, :], x_view,
                                         y_t[:, k, 0:rc, :])
                    for j2 in range(rc // 2):
                        r0, r1 = 2 * j2, 2 * j2 + 2
                        nc.tensor.matmul(
                            ps1[:, r0:r1, :], lhsT=id_sb[:],
                            rhs=p_t[:, r0:r1, :],
                            start=(k == 0), stop=(k == 8))
                nc.scalar.copy(o1_sb[:, q0:q0 + rc, 1:W + 1], ps1[:, 0:rc, :])

            def conv_chunk(j):
                m0 = RC * j
                ps2 = ps2_pool.tile([128, RC, W], F32, name="ps2", tag="ps2")
                for t in range(9):
                    i3, j3 = divmod(t, 3)
                    for j2 in (0, 1):
                        r0 = 2 * j2
                        nc.tensor.matmul(
                            ps2[:, r0:r0 + 2, :], lhsT=w_sb[:, t],
                            rhs=o1_sb[:, m0 + i3 + r0: m0 + i3 + r0 + 2,
                                      j3: j3 + W],
                            start=(t == 0), stop=(t == 8))
                # leaky_relu(v) = max(v, 0.2*v), v = conv + b
                v_t = v_pool.tile([128, RC, W], F32, name="v_t", tag="v_t")
                nc.vector.tensor_scalar_add(v_t[:], ps2[:], b_sb[:, 0:1])
                o_t = o_pool.tile([128, RC, W], F32, name="o_t", tag="o_t")
                nc.vector.scalar_tensor_tensor(
                    o_t[:], v_t[:], 0.2, v_t[:],
                    mybir.AluOpType.mult, mybir.AluOpType.max)
                for s in (0, 1):
                    nc.sync.dma_start(
                        out_d[:, HH * s + RC * j: HH * s + RC * j + RC, :],
                        o_t[64 * s:64 * s + 64])

            def body():
                for c in range(NRED):
                    reduce_chunk(c)
                    if c >= 1:
                        conv_chunk(c - 1)

            if loop_reps is None:
                body()
            else:
                with tc.For_i(0, loop_reps, 1,
                              hint_engines=(mybir.EngineType.PE,)):
                    body()

    nc.compile()
    return nc


_PROGRAM = None


def _get_program():
    global _PROGRAM
    if _PROGRAM is None:
        _PROGRAM = _build_program()
    return _PROGRAM


def make_in_maps(x, y, fuse_w, fuse_b):
    x = np.asarray(x, dtype=np.float32)
    y = np.asarray(y, dtype=np.float32)
    fuse_w = np.asarray(fuse_w, dtype=np.float32)
    fuse_b = np.asarray(fuse_b, dtype=np.float32)

    # block-diagonal conv weights: each partition half (h-half of the
    # slab) contracts with its own copy of W_tap in one K=128 matmul
    wt = np.zeros((9, 128, 128), np.float32)
    for t in range(9):
        i, j = divmod(t, 3)
        wtap = fuse_w[:, :, i, j].T  # [c_in, c_out]
        wt[t, 0:64, 0:64] = wtap
        wt[t, 64:128, 64:128] = wtap
    ident = np.eye(128, dtype=np.float32)
    bias = np.concatenate([fuse_b, fuse_b]).astype(np.float32)[:, None]

    in_maps = []
    for core in range(NCORES):
        n, hb = divmod(core, 4)
        h0 = hb * HB
        xp = np.zeros((C, 70, XW), np.float32)
        r0, r1 = h0 - 3, h0 + 67
        s0, s1 = max(r0, 0), min(r1, H)
        xp[:, s0 - r0:s1 - r0, D:D + W] = x[n, :, s0:s1, :]
        yp = np.zeros((C * 9, 66, W), np.float32)
        r0y, r1y = h0 - 1, h0 + 65
        s0y, s1y = max(r0y, 0), min(r1y, H)
        yp[:, s0y - r0y:s1y - r0y, :] = y[n, :, s0y:s1y, :]
        in_maps.append({"xp": xp, "yp": yp, "wt": wt, "ident": ident,
                        "bias": bias})
    return in_maps


def run(x, y, fuse_w, fuse_b, trace=False, **kw):
    nc = _get_program()
    in_maps = make_in_maps(x, y, fuse_w, fuse_b)
    res = run_bass_kernel_spmd(nc, in_maps, list(range(NCORES)),
                               trace=trace, **kw)
    out = np.empty((N, C, H, W), np.float32)
    for core in range(NCORES):
        n, hb = divmod(core, 4)
        out[n, :, hb * HB:(hb + 1) * HB, :] = res.results[core]["out"]
    return out, res


def kernel(x, y, fuse_w, fuse_b):
    out, _ = run(x, y, fuse_w, fuse_b, trace=False)
    return out



# revision 2
# speedup vs baseline: 2.4533x; 2.4533x over previous
"""DepthDC fused kernel for 8 Trainium2 NeuronCores.

Reference computation (N=2, C=64, H=W=256, d=2):
  patches[n,c,k,h,w] = xpad[n,c,h+ki*d, w+kj*d]   (k=3*ki+kj, pad d)
  out1 = sum_k patches * y.reshape(N,C,9,H,W)
  out  = leaky_relu(conv3x3(out1, fuse_w) + fuse_b, 0.2)

Sharding: 8 cores = batch(2) x H-quarters(4). Each core produces a
[64, 64, 256] output slab. Host slices overlapping (haloed, zero-padded)
input slabs per core, so no device collectives are needed.

Per-core layout: the 64 output rows split into two 32-row halves mapped
to SBUF partition halves (partition = c + 64*s). All on-chip data is
fp16 (PSUM accumulation in fp32); the host casts inputs to fp16 and the
fp16 output back to fp32. y is host-packed as [(s c), q, k, w] so each
per-chunk DMA is one 18KB-contiguous descriptor per partition.

Engines:
  - DVE: 9 elementwise products per chunk (fp16, 2x mode)
  - k-reduction: per-chunk either PE (identity matmul, PSUM acc) or
    DVE (tree of fp16 adds), set by REDUCE_MODE to balance engine load
  - PE:  3x3 dense conv as 9 accumulating matmuls over C=64 (fp16)
  - ACT: PSUM->SBUF out1 copies (PE-mode chunks) and the whole
    bias+leaky-relu epilogue as one Prelu activation per chunk
Work is streamed over 4-row h-chunks with triple-buffered y DMA.
"""

import sys

sys.path.insert(0, "/opt/trn_rl_repo")

from contextlib import ExitStack

import numpy as np

import concourse.bass as bass
import concourse.mybir as mybir
import concourse.tile as tile
from concourse import bacc
from concourse.bass_utils import run_bass_kernel_spmd

F16 = mybir.dt.float16
F32 = mybir.dt.float32
AF = mybir.ActivationFunctionType

N, C, H, W = 2, 64, 256, 256
D = 2  # dilation == pad
NEG_SLOPE = 0.2
NCORES = 8
HB = 64          # output rows per core
HH = 32          # output rows per half
Q = HH + 2       # out1 rows per half (34)
XR = Q + 4       # x rows per half block (38)
XW = W + 2 * D   # padded x width (260)
OW = W + 2       # padded out1 width (258)
RC = 4           # rows per chunk
NRED = 9         # reduce chunks per half: 8 x 4 rows + 1 x 2 rows
NCONV = 8        # conv chunks per half: 8 x 4 rows

# Which engine reduces the 9 products per chunk: "pe" = identity matmul
# with PSUM accumulation, "dve" = tree of fp16 adds on DVE.
REDUCE_MODE = ["dve", "pe", "dve", "pe", "dve", "pe", "dve", "pe", "dve"]
# Skip the per-matmul LDWEIGHTS when consecutive matmuls share lhsT.
ELIDE_LDW = False


def _build_program():
    nc = bacc.Bacc("TRN2", target_bir_lowering=False, debug=False,
                   num_devices=NCORES)

    xp_d = nc.dram_tensor("xp", [128, XR, XW], F16, kind="ExternalInput").ap()
    yp_d = nc.dram_tensor("yp", [128, Q, 9, W], F16,
                          kind="ExternalInput").ap()
    wt_d = nc.dram_tensor("wt", [9, 128, 128], F16, kind="ExternalInput").ap()
    id_d = nc.dram_tensor("ident", [128, 128], F16, kind="ExternalInput").ap()
    b_d = nc.dram_tensor("bias", [128, 1], F32, kind="ExternalInput").ap()
    out_d = nc.dram_tensor("out", [128, HH, W], F16, kind="ExternalOutput").ap()

    with tile.TileContext(nc) as tc:
        with ExitStack() as ctx:
            const = ctx.enter_context(tc.tile_pool(name="const", bufs=1))
            y_pool = ctx.enter_context(tc.tile_pool(name="y_pool", bufs=3))
            p_pool = ctx.enter_context(tc.tile_pool(name="p_pool", bufs=4))
            a_pool = ctx.enter_context(tc.tile_pool(name="a_pool", bufs=2))
            ps1_pool = ctx.enter_context(
                tc.tile_pool(name="ps1_pool", bufs=2, space="PSUM"))
            ps2_pool = ctx.enter_context(
                tc.tile_pool(name="ps2_pool", bufs=2, space="PSUM"))

            # constants / whole-slab x / whole-slab out1 / whole-slab out
            w_sb = const.tile([128, 9, 128], F16, name="w_sb")
            nc.sync.dma_start(w_sb[:], wt_d.rearrange("t p m -> p t m"))
            id_sb = const.tile([128, 128], F16, name="id_sb")
            nc.sync.dma_start(id_sb[:], id_d)
            b_sb = const.tile([128, 1], F32, name="b_sb")
            nc.sync.dma_start(b_sb[:], b_d)
            x_sb = const.tile([128, XR, XW], F16, name="x_sb")
            nc.sync.dma_start(x_sb[:], xp_d)
            o1_sb = const.tile([128, Q, OW], F16, name="o1_sb")
            o_all = const.tile([128, HH, W], F16, name="o_all")
            # zero the conv W-padding columns once (ACT, scale=0 writes 0)
            nc.scalar.activation(o1_sb[:, :, 0:1], o1_sb[:, :, 0:1],
                                 AF.Copy, scale=0.0)
            nc.scalar.activation(o1_sb[:, :, OW - 1:OW],
                                 o1_sb[:, :, OW - 1:OW], AF.Copy, scale=0.0)
            # Wait-merge scratch: one cheap DVE copy per input DMA converts
            # DMA-completion semaphores into DVE program order, so compute
            # instructions never need more than 1 foreign wait sem (the
            # TT-struct wait-slot limit in walrus codegen is tight).
            scr = const.tile([128, 8], F16, name="scr")
            nc.vector.tensor_copy(scr[:, 0:1], x_sb[:, 0, 0:1])
            nc.vector.tensor_copy(scr[:, 1:2], x_sb[:, XR - 1, 0:1])
            nc.vector.tensor_copy(scr[:, 2:3], w_sb[:, 0, 0:1])
            nc.vector.tensor_copy(scr[:, 3:4], id_sb[:, 0:1])
            nc.vector.tensor_copy(scr[:, 4:5],
                                  b_sb[:, 0:1].bitcast(F16)[:, 0:1])

            def x_view(k, q0, rc):
                ki, kj = divmod(k, 3)
                return x_sb[:, q0 + 2 * ki: q0 + 2 * ki + rc,
                            2 * kj: 2 * kj + W]

            def reduce_chunk(c):
                q0 = RC * c
                rc = min(RC, Q - q0)
                y_t = y_pool.tile([128, RC, 9, W], F16, name="y_t", tag="y_t")
                nc.sync.dma_start(y_t[:, 0:rc], yp_d[:, q0:q0 + rc])
                nc.vector.tensor_copy(scr[:, 5:6], y_t[:, 0, 0, 0:1])
                if REDUCE_MODE[c] == "pe":
                    ps1 = ps1_pool.tile([128, RC, W], F32, name="ps1",
                                        tag="ps1")
                    mms = []
                    if ELIDE_LDW:
                        # all products first, then a clean run of matmuls
                        # sharing the preloaded identity weights
                        p_ts = []
                        for k in range(9):
                            p_t = p_pool.tile([128, RC, W], F16, name="p_t",
                                              tag="p_t")
                            nc.vector.tensor_mul(p_t[:, 0:rc],
                                                 x_view(k, q0, rc),
                                                 y_t[:, 0:rc, k, :])
                            p_ts.append(p_t)
                        for k in range(9):
                            for g in range(rc // 2):
                                r0 = 2 * g
                                mms.append(nc.tensor.matmul(
                                    ps1[:, r0:r0 + 2, :], lhsT=id_sb[:],
                                    rhs=p_ts[k][:, r0:r0 + 2, :],
                                    start=(k == 0), stop=(k == 8)))
                        for mm in mms[1:]:
                            mm.ldweights = False
                    else:
                        for k in range(9):
                            p_t = p_pool.tile([128, RC, W], F16, name="p_t",
                                              tag="p_t")
                            nc.vector.tensor_mul(p_t[:, 0:rc],
                                                 x_view(k, q0, rc),
                                                 y_t[:, 0:rc, k, :])
                            for g in range(rc // 2):
                                r0 = 2 * g
                                nc.tensor.matmul(
                                    ps1[:, r0:r0 + 2, :], lhsT=id_sb[:],
                                    rhs=p_t[:, r0:r0 + 2, :],
                                    start=(k == 0), stop=(k == 8))
                    nc.scalar.copy(o1_sb[:, q0:q0 + rc, 1:W + 1],
                                   ps1[:, 0:rc, :])
                else:
                    acc = a_pool.tile([128, RC, W], F16, name="acc",
                                      tag="acc")
                    p0 = p_pool.tile([128, RC, W], F16, name="p_t", tag="p_t")
                    nc.vector.tensor_mul(p0[:, 0:rc], x_view(0, q0, rc),
                                         y_t[:, 0:rc, 0, :])
                    p1 = p_pool.tile([128, RC, W], F16, name="p_t", tag="p_t")
                    nc.vector.tensor_mul(p1[:, 0:rc], x_view(1, q0, rc),
                                         y_t[:, 0:rc, 1, :])
                    nc.vector.tensor_add(acc[:, 0:rc], p0[:, 0:rc],
                                         p1[:, 0:rc])
                    for k in range(2, 9):
                        p_t = p_pool.tile([128, RC, W], F16, name="p_t",
                                          tag="p_t")
                        nc.vector.tensor_mul(p_t[:, 0:rc], x_view(k, q0, rc),
                                             y_t[:, 0:rc, k, :])
                        if k < 8:
                            nc.vector.tensor_add(acc[:, 0:rc], acc[:, 0:rc],
                                                 p_t[:, 0:rc])
                        else:
                            nc.vector.tensor_add(
                                o1_sb[:, q0:q0 + rc, 1:W + 1],
                                acc[:, 0:rc], p_t[:, 0:rc])

            def conv_chunk(j):
                m0 = RC * j
                ps2 = ps2_pool.tile([128, RC, W], F32, name="ps2", tag="ps2")
                mms_by_tap = []
                for t in range(9):
                    i3, j3 = divmod(t, 3)
                    tap_mms = []
                    for g in (0, 1):
                        r0 = 2 * g
                        tap_mms.append(nc.tensor.matmul(
                            ps2[:, r0:r0 + 2, :], lhsT=w_sb[:, t],
                            rhs=o1_sb[:, m0 + i3 + r0: m0 + i3 + r0 + 2,
                                      j3: j3 + W],
                            start=(t == 0), stop=(t == 8)))
                    mms_by_tap.append(tap_mms)
                if ELIDE_LDW:
                    for tap_mms in mms_by_tap:
                        for mm in tap_mms[1:]:
                            mm.ldweights = False
                # whole epilogue on ACT: prelu(v + b, 0.2), v = conv psum
                nc.scalar.activation(o_all[:, m0:m0 + RC, :], ps2[:],
                                     AF.Prelu, bias=b_sb[:, 0:1], scale=1.0,
                                     alpha=NEG_SLOPE)
                if j % 2 == 1:
                    g0 = m0 - RC
                    nc.sync.dma_start(out_d[:, g0:g0 + 2 * RC, :],
                                      o_all[:, g0:g0 + 2 * RC, :])

            for c in range(NRED):
                reduce_chunk(c)
                if c >= 1:
                    conv_chunk(c - 1)

    nc.compile()
    return nc


_PROGRAM = None


def _get_program():
    global _PROGRAM
    if _PROGRAM is None:
        _PROGRAM = _build_program()
    return _PROGRAM


def make_in_maps(x, y, fuse_w, fuse_b):
    x = np.asarray(x, dtype=np.float32)
    y = np.asarray(y, dtype=np.float32)
    fuse_w = np.asarray(fuse_w, dtype=np.float32)
    fuse_b = np.asarray(fuse_b, dtype=np.float32)

    # x padded to fp16 [N, C, H+6, W+4]: row offset +3, col offset +2
    xf = np.zeros((N, C, H + 6, W + 4), np.float16)
    xf[:, :, 3:3 + H, 2:2 + W] = x
    # y as fp16 [N, C, 9, H+2, W]: row offset +1
    yf = np.zeros((N, C, 9, H + 2, W), np.float16)
    yf[:, :, :, 1:1 + H, :] = y.reshape(N, C, 9, H, W)

    # block-diagonal conv weights: each partition half (h-half of the
    # slab) contracts with its own copy of W_tap in one K=128 matmul
    wt = np.zeros((9, 128, 128), np.float16)
    for t in range(9):
        i, j = divmod(t, 3)
        wtap = fuse_w[:, :, i, j].T  # [c_in, c_out]
        wt[t, 0:64, 0:64] = wtap
        wt[t, 64:128, 64:128] = wtap
    ident = np.eye(128, dtype=np.float16)
    bias = np.concatenate([fuse_b, fuse_b]).astype(np.float32)[:, None]

    in_maps = []
    for core in range(NCORES):
        n, hb = divmod(core, 4)
        h0 = hb * HB
        # x slab [(s c), 38, 260]
        xp = np.concatenate(
            [xf[n, :, h0:h0 + XR, :], xf[n, :, h0 + HH:h0 + HH + XR, :]],
            axis=0)
        xp = np.ascontiguousarray(xp)
        # y slab [(s c), 34, 9, 256]: rows h0+32s-1+q (offset +1 in yf)
        yhalves = []
        for s in (0, 1):
            r0 = h0 + HH * s
            yh = yf[n, :, :, r0:r0 + Q, :]          # [C, 9, Q, W]
            yhalves.append(yh.transpose(0, 2, 1, 3))  # [C, Q, 9, W]
        yp = np.ascontiguousarray(np.concatenate(yhalves, axis=0))
        in_maps.append({"xp": xp, "yp": yp, "wt": wt, "ident": ident,
                        "bias": bias})
    return in_maps


def run(x, y, fuse_w, fuse_b, trace=False, **kw):
    nc = _get_program()
    in_maps = make_in_maps(x, y, fuse_w, fuse_b)
    res = run_bass_kernel_spmd(nc, in_maps, list(range(NCORES)),
                               trace=trace, **kw)
    out = np.empty((N, C, H, W), np.float32)
    for core in range(NCORES):
        n, hb = divmod(core, 4)
        h0 = hb * HB
        r = np.asarray(res.results[core]["out"], dtype=np.float32)
        out[n, :, h0:h0 + HH, :] = r[0:64]
        out[n, :, h0 + HH:h0 + HB, :] = r[64:128]
    return out, res


def kernel(x, y, fuse_w, fuse_b):
    out, _ = run(x, y, fuse_w, fuse_b, trace=False)
    return out


# revision 7
# speedup vs baseline: 2.6854x; 1.0946x over previous
"""DepthDC fused kernel for 8 Trainium2 NeuronCores.

Reference computation (N=2, C=64, H=W=256, d=2):
  patches[n,c,k,h,w] = xpad[n,c,h+ki*d, w+kj*d]   (k=3*ki+kj, pad d)
  out1 = sum_k patches * y.reshape(N,C,9,H,W)
  out  = leaky_relu(conv3x3(out1, fuse_w) + fuse_b, 0.2)

Sharding: 8 cores = batch(2) x H-quarters(4). Each core produces a
[64, 64, 256] output slab. Host slices overlapping (haloed, zero-padded)
input slabs per core, so no device collectives are needed.

Per-core layout: the 64 output rows split into two 32-row halves mapped
to SBUF partition halves (partition = c + 64*s). All on-chip data is
fp16 (PSUM accumulation in fp32); the host casts inputs to fp16 and the
fp16 output back to fp32. y is host-packed as [(s c), q, k, w] so each
per-chunk DMA is one 18KB-contiguous descriptor per partition.

Engines:
  - DVE: 9 elementwise products per chunk (fp16, 2x mode)
  - k-reduction: per-chunk either PE (identity matmul, PSUM acc) or
    DVE (tree of fp16 adds), set by REDUCE_MODE to balance engine load
  - PE:  3x3 dense conv as 9 accumulating matmuls over C=64 (fp16)
  - ACT: PSUM->SBUF out1 copies (PE-mode chunks) and the whole
    bias+leaky-relu epilogue as one Prelu activation per chunk
Work is streamed over 4-row h-chunks with triple-buffered y DMA.
"""

import sys

sys.path.insert(0, "/opt/trn_rl_repo")

from contextlib import ExitStack

import numpy as np

import concourse.bass as bass
import concourse.mybir as mybir
import concourse.tile as tile
from concourse import bacc
from concourse.bass_utils import run_bass_kernel_spmd

F16 = mybir.dt.float16
F32 = mybir.dt.float32
AF = mybir.ActivationFunctionType

N, C, H, W = 2, 64, 256, 256
D = 2  # dilation == pad
NEG_SLOPE = 0.2
NCORES = 8
HB = 64          # output rows per core
HH = 32          # output rows per half
Q = HH + 2       # out1 rows per half (34)
XR = Q + 4       # x rows per half block (38)
XW = W + 2 * D   # padded x width (260)
OW = W + 2       # padded out1 width (258)
RC = 4           # rows per chunk
NRED = 9         # reduce chunks per half: 8 x 4 rows + 1 x 2 rows
NCONV = 8        # conv chunks per half: 8 x 4 rows

# Which engine reduces the 9 products per chunk: "pe" = identity matmul
# with PSUM accumulation, "dve" = tree of fp16 adds on DVE.
REDUCE_MODE = ["dve", "pe", "dve", "pe", "dve", "pe", "dve", "pe", "dve"]
# Skip the per-matmul LDWEIGHTS when consecutive matmuls share lhsT.
ELIDE_LDW = False


def _build_program():
    nc = bacc.Bacc("TRN2", target_bir_lowering=False, debug=False,
                   num_devices=NCORES)

    xp_d = nc.dram_tensor("xp", [128, XR, XW], F16, kind="ExternalInput").ap()
    yp_d = nc.dram_tensor("yp", [128, Q, 9, W], F16,
                          kind="ExternalInput").ap()
    wt_d = nc.dram_tensor("wt", [9, 128, 128], F16, kind="ExternalInput").ap()
    id_d = nc.dram_tensor("ident", [128, 128], F16, kind="ExternalInput").ap()
    b_d = nc.dram_tensor("bias", [128, 1], F32, kind="ExternalInput").ap()
    out_d = nc.dram_tensor("out", [128, HH, W], F16, kind="ExternalOutput").ap()

    with tile.TileContext(nc) as tc:
        with ExitStack() as ctx:
            const = ctx.enter_context(tc.tile_pool(name="const", bufs=1))
            y_pool = ctx.enter_context(tc.tile_pool(name="y_pool", bufs=3))
            p_pool = ctx.enter_context(tc.tile_pool(name="p_pool", bufs=20))
            a_pool = ctx.enter_context(tc.tile_pool(name="a_pool", bufs=2))
            ps1_pool = ctx.enter_context(
                tc.tile_pool(name="ps1_pool", bufs=2, space="PSUM"))
            ps2_pool = ctx.enter_context(
                tc.tile_pool(name="ps2_pool", bufs=2, space="PSUM"))

            # constants / whole-slab x / whole-slab out1 / whole-slab out
            w_sb = const.tile([128, 9, 128], F16, name="w_sb")
            nc.sync.dma_start(w_sb[:], wt_d.rearrange("t p m -> p t m"))
            id_sb = const.tile([128, 128], F16, name="id_sb")
            nc.sync.dma_start(id_sb[:], id_d)
            b_sb = const.tile([128, 1], F32, name="b_sb")
            nc.sync.dma_start(b_sb[:], b_d)
            x_sb = const.tile([128, XR, XW], F16, name="x_sb")
            nc.sync.dma_start(x_sb[:], xp_d)
            o1_sb = const.tile([128, Q, OW], F16, name="o1_sb")
            o_all = const.tile([128, HH, W], F16, name="o_all")
            # zero the conv W-padding columns once (ACT, scale=0 writes 0)
            nc.scalar.activation(o1_sb[:, :, 0:1], o1_sb[:, :, 0:1],
                                 AF.Copy, scale=0.0)
            nc.scalar.activation(o1_sb[:, :, OW - 1:OW],
                                 o1_sb[:, :, OW - 1:OW], AF.Copy, scale=0.0)
            # Wait-merge scratch: one cheap DVE copy per input DMA converts
            # DMA-completion semaphores into DVE program order, so compute
            # instructions never need more than 1 foreign wait sem (the
            # TT-struct wait-slot limit in walrus codegen is tight).
            scr = const.tile([128, 8], F16, name="scr")
            nc.vector.tensor_copy(scr[:, 0:1], x_sb[:, 0, 0:1])
            nc.vector.tensor_copy(scr[:, 1:2], x_sb[:, XR - 1, 0:1])
            nc.vector.tensor_copy(scr[:, 2:3], w_sb[:, 0, 0:1])
            nc.vector.tensor_copy(scr[:, 3:4], id_sb[:, 0:1])
            nc.vector.tensor_copy(scr[:, 4:5],
                                  b_sb[:, 0:1].bitcast(F16)[:, 0:1])

            def x_view(k, q0, rc):
                ki, kj = divmod(k, 3)
                return x_sb[:, q0 + 2 * ki: q0 + 2 * ki + rc,
                            2 * kj: 2 * kj + W]

            def reduce_chunk(c):
                q0 = RC * c
                rc = min(RC, Q - q0)
                y_t = y_pool.tile([128, RC, 9, W], F16, name="y_t", tag="y_t")
                nc.sync.dma_start(y_t[:, 0:rc], yp_d[:, q0:q0 + rc])
                nc.vector.tensor_copy(scr[:, 5:6], y_t[:, 0, 0, 0:1])
                if REDUCE_MODE[c] == "pe":
                    ps1 = ps1_pool.tile([128, RC, W], F32, name="ps1",
                                        tag="ps1")
                    # all products first, then a clean run of matmuls so
                    # the PE sees a dense stream once products are ready
                    p_ts = []
                    for k in range(9):
                        p_t = p_pool.tile([128, RC, W], F16, name="p_t",
                                          tag="p_t")
                        nc.vector.tensor_mul(p_t[:, 0:rc],
                                             x_view(k, q0, rc),
                                             y_t[:, 0:rc, k, :])
                        p_ts.append(p_t)
                    mms = []
                    for k in range(9):
                        for g in range(rc // 2):
                            r0 = 2 * g
                            mms.append(nc.tensor.matmul(
                                ps1[:, r0:r0 + 2, :], lhsT=id_sb[:],
                                rhs=p_ts[k][:, r0:r0 + 2, :],
                                start=(k == 0), stop=(k == 8)))
                    if ELIDE_LDW:
                        for mm in mms[1:]:
                            mm.ldweights = False
                    nc.scalar.copy(o1_sb[:, q0:q0 + rc, 1:W + 1],
                                   ps1[:, 0:rc, :])
                else:
                    acc = a_pool.tile([128, RC, W], F16, name="acc",
                                      tag="acc")
                    p0 = p_pool.tile([128, RC, W], F16, name="p_t", tag="p_t")
                    nc.vector.tensor_mul(p0[:, 0:rc], x_view(0, q0, rc),
                                         y_t[:, 0:rc, 0, :])
                    p1 = p_pool.tile([128, RC, W], F16, name="p_t", tag="p_t")
                    nc.vector.tensor_mul(p1[:, 0:rc], x_view(1, q0, rc),
                                         y_t[:, 0:rc, 1, :])
                    nc.vector.tensor_add(acc[:, 0:rc], p0[:, 0:rc],
                                         p1[:, 0:rc])
                    for k in range(2, 9):
                        p_t = p_pool.tile([128, RC, W], F16, name="p_t",
                                          tag="p_t")
                        nc.vector.tensor_mul(p_t[:, 0:rc], x_view(k, q0, rc),
                                             y_t[:, 0:rc, k, :])
                        if k < 8:
                            nc.vector.tensor_add(acc[:, 0:rc], acc[:, 0:rc],
                                                 p_t[:, 0:rc])
                        else:
                            nc.vector.tensor_add(
                                o1_sb[:, q0:q0 + rc, 1:W + 1],
                                acc[:, 0:rc], p_t[:, 0:rc])

            def conv_chunk(j):
                m0 = RC * j
                ps2 = ps2_pool.tile([128, RC, W], F32, name="ps2", tag="ps2")
                mms_by_tap = []
                for t in range(9):
                    i3, j3 = divmod(t, 3)
                    tap_mms = []
                    for g in (0, 1):
                        r0 = 2 * g
                        tap_mms.append(nc.tensor.matmul(
                            ps2[:, r0:r0 + 2, :], lhsT=w_sb[:, t],
                            rhs=o1_sb[:, m0 + i3 + r0: m0 + i3 + r0 + 2,
                                      j3: j3 + W],
                            start=(t == 0), stop=(t == 8)))
                    mms_by_tap.append(tap_mms)
                if ELIDE_LDW:
                    for tap_mms in mms_by_tap:
                        for mm in tap_mms[1:]:
                            mm.ldweights = False
                # whole epilogue on ACT: prelu(v + b, 0.2), v = conv psum
                nc.scalar.activation(o_all[:, m0:m0 + RC, :], ps2[:],
                                     AF.Prelu, bias=b_sb[:, 0:1], scale=1.0,
                                     alpha=NEG_SLOPE)
                if j % 2 == 1:
                    g0 = m0 - RC
                    nc.sync.dma_start(out_d[:, g0:g0 + 2 * RC, :],
                                      o_all[:, g0:g0 + 2 * RC, :])

            # conv trails the reduce by 2 chunks so its o1 rows (and the
            # ACT PSUM->SBUF copy that produces them) are long since done
            # when the PE reaches the conv matmuls — no mid-stream stalls.
            for c in range(NRED):
                reduce_chunk(c)
                if c >= 2:
                    conv_chunk(c - 2)
            conv_chunk(NCONV - 1)

    nc.compile()
    return nc


_PROGRAM = None


def _get_program():
    global _PROGRAM
    if _PROGRAM is None:
        _PROGRAM = _build_program()
    return _PROGRAM


def make_in_maps(x, y, fuse_w, fuse_b):
    x = np.asarray(x, dtype=np.float32)
    y = np.asarray(y, dtype=np.float32)
    fuse_w = np.asarray(fuse_w, dtype=np.float32)
    fuse_b = np.asarray(fuse_b, dtype=np.float32)

    # x padded to fp16 [N, C, H+6, W+4]: row offset +3, col offset +2
    xf = np.zeros((N, C, H + 6, W + 4), np.float16)
    xf[:, :, 3:3 + H, 2:2 + W] = x
    # y as fp16 [N, C, 9, H+2, W]: row offset +1
    yf = np.zeros((N, C, 9, H + 2, W), np.float16)
    yf[:, :, :, 1:1 + H, :] = y.reshape(N, C, 9, H, W)

    # block-diagonal conv weights: each partition half (h-half of the
    # slab) contracts with its own copy of W_tap in one K=128 matmul
    wt = np.zeros((9, 128, 128), np.float16)
    for t in range(9):
        i, j = divmod(t, 3)
        wtap = fuse_w[:, :, i, j].T  # [c_in, c_out]
        wt[t, 0:64, 0:64] = wtap
        wt[t, 64:128, 64:128] = wtap
    ident = np.eye(128, dtype=np.float16)
    bias = np.concatenate([fuse_b, fuse_b]).astype(np.float32)[:, None]

    in_maps = []
    for core in range(NCORES):
        n, hb = divmod(core, 4)
        h0 = hb * HB
        # x slab [(s c), 38, 260]
        xp = np.concatenate(
            [xf[n, :, h0:h0 + XR, :], xf[n, :, h0 + HH:h0 + HH + XR, :]],
            axis=0)
        xp = np.ascontiguousarray(xp)
        # y slab [(s c), 34, 9, 256]: rows h0+32s-1+q (offset +1 in yf)
        yhalves = []
        for s in (0, 1):
            r0 = h0 + HH * s
            yh = yf[n, :, :, r0:r0 + Q, :]          # [C, 9, Q, W]
            yhalves.append(yh.transpose(0, 2, 1, 3))  # [C, Q, 9, W]
        yp = np.ascontiguousarray(np.concatenate(yhalves, axis=0))
        in_maps.append({"xp": xp, "yp": yp, "wt": wt, "ident": ident,
                        "bias": bias})
    return in_maps


def run(x, y, fuse_w, fuse_b, trace=False, **kw):
    nc = _get_program()
    in_maps = make_in_maps(x, y, fuse_w, fuse_b)
    res = run_bass_kernel_spmd(nc, in_maps, list(range(NCORES)),
                               trace=trace, **kw)
    out = np.empty((N, C, H, W), np.float32)
    for core in range(NCORES):
        n, hb = divmod(core, 4)
        h0 = hb * HB
        r = np.asarray(res.results[core]["out"], dtype=np.float32)
        out[n, :, h0:h0 + HH, :] = r[0:64]
        out[n, :, h0 + HH:h0 + HB, :] = r[64:128]
    return out, res


def kernel(x, y, fuse_w, fuse_b):
    out, _ = run(x, y, fuse_w, fuse_b, trace=False)
    return out
